# revision 10
# baseline (speedup 1.0000x reference)
"""Block-3D attention kernel for 8 Trainium2 NeuronCores.

Problem: B=2, 16x16x16 token grid, 8x8x8 blocks -> 16 independent blocks
of T=512 tokens. GQA attention (32 q heads, 8 kv heads, d=64) inside each
block, with QKV/O projections (hidden=2048).

Sharding: pure data-parallel over blocks - 2 blocks per core, full
weights replicated, no collectives. Each core runs an identical program
on its own slice.

Per-core dataflow (all matmuls bf16 with fp32 PSUM accumulation):
  hbT [2048,1024] (hidden, block-permuted, transposed, bf16)
  1. Q/K projections, weights stationary -> qT [2048,1024], kTdup
     (kv heads duplicated on both partition halves for 2-head row-tiled QK)
  2. V projection, activations stationary -> v [t, kv*64] (+ones cols)
  3. per (block, head-pair): st[s,t] = k q^T via two row-tiled matmuls;
     exp on ACT -> pT bf16
  4. PV: lhsT=pT chunks, rhs=[v|1] -> o[t, 65] psum; col 64 = sum(exp);
     vector reciprocal + per-partition tensor_scalar_mul -> o_all [t, hd]
  5. PE-transpose o_all -> oT [hd, t]
  6. Wo: lhsT=woT tiles, rhs=oT -> out^T [2048, 1024] f32
"""

import numpy as np
import ml_dtypes

import concourse.bass as bass
import concourse.mybir as mybir
from concourse.tile import TileContext
from concourse.masks import make_identity
from concourse.bass_utils import run_bass_kernel_spmd

# ---------------------------------------------------------------------------
# Workaround for this walrus build: at most 1 sync wait per Drain
# instruction, but TileContext's tail drain collects one wait per active
# proc. Split the waits across per-proc NOPs on the sync engine.
# ---------------------------------------------------------------------------
from concourse import tile as _tile
from concourse.vector_clock import ScopedClock as _ScopedClock
from concourse.vector_clock import VectorClock as _VectorClock
from concourse.tile_sem_assignment import N_PROCS as _N_PROCS


def _split_drain_and_barrier(self, tick_clock, wait_clock):
    gc = tick_clock.global_clock
    for p in range(_N_PROCS):
        if gc[p] == 0:
            continue
        c = _VectorClock([gc[q] if q == p else 0 for q in range(_N_PROCS)])
        nop = self.nc.sync.nop(nofuse=True)
        wait_clock.add_sem_waits(nop.ins, _ScopedClock({None: c}))
    # The NOPs above precede the drain in SP program order and carry all
    # required waits, so the drain itself needs none.
    self.nc.sync.drain()
    self.nc.all_engine_barrier()
    assert self.sems is not None
    popped = self.nc._tile_sem_poison_stack.pop()
    assert popped is self._sem_poison
    self.nc.clear_and_free_semaphores(list(self.sems.allocated().values()))
    self.nc.all_engine_barrier()


_tile.TileContext._drain_and_barrier = _split_drain_and_barrier

# This walrus also caps sync waits per regular instruction (observed: 3
# waits on a DVE TensorCopy rejected). Post-pass: move excess waits onto
# bass_nofuse NOPs inserted immediately before the instruction on the
# same engine.
_WAIT_CAP = 1

from concourse.tile_rust import add_dep_helper as _add_dep_helper


def _add_dep(from_inst, to_inst, reason=""):
    _add_dep_helper(from_inst, to_inst, sync=False, reason=reason)


def _act_reciprocal(nc, out, in_):
    """Reciprocal on the Scalar (ACT) engine. bass blocks
    ActivationFunctionType.Reciprocal for accuracy; measured on this HW the
    rel err is ~1.2e-5 for inputs in [300, 2500] (our softmax denominators),
    far below this kernel's bf16-dominated error floor, and it is ~5x
    cheaper than the exact DVE reciprocal at free size 512."""
    eng = nc.scalar
    return eng.add_instruction(
        mybir.InstActivation(
            name=nc.get_next_instruction_name(),
            func=mybir.ActivationFunctionType.Reciprocal,
            ins=[eng.lower_ap(in_),
                 mybir.ImmediateValue(dtype=mybir.dt.float32, value=0.0),
                 mybir.ImmediateValue(dtype=mybir.dt.float32, value=1.0),
                 mybir.ImmediateValue(dtype=mybir.dt.float32, value=0.0)],
            outs=[eng.lower_ap(out)],
        )
    )


def _split_excess_waits(nc, cap=_WAIT_CAP):
    count = 0
    for f in nc.m.functions:
        for bb in f.blocks:
            il = bb.instructions
            i = 0
            while i < len(il):
                inst = il[i]
                si = inst.sync_info
                c = 1 if isinstance(inst, mybir.InstDrain) else cap
                if si is not None and len(si.on_wait) > c:
                    waits = list(si.on_wait)
                    keep = waits[-c:] if c else []
                    excess = waits[:-c] if c else waits
                    pos = i
                    for g0 in range(0, len(excess), cap):
                        grp = excess[g0:g0 + cap]
                        count += 1
                        nop = mybir.InstNoOp(
                            name=f"waitsplit_{count}",
                            sync_info=mybir.SyncInfo(on_wait=grp, on_update=[]),
                            bass_nofuse=True,
                            engine=inst.engine,
                        )
                        il.insert(pos, nop)
                        pos += 1
                        i += 1
                    si.on_wait = keep
                i += 1
    return count

# ---------------------------------------------------------------------------
# Model constants (hardcoded per problem spec)
# ---------------------------------------------------------------------------
HID = 2048
NH = 32
NKV = 8
D = 64
B = 2
GRID = 16           # x_dim = y_dim = z_dim
BS = 8              # block size per axis
T = BS * BS * BS    # 512 tokens per block
NBLOCKS = 16        # total 3D blocks (B * 2*2*2)
N_CORES = 8
BPC = NBLOCKS // N_CORES  # blocks per core = 2
TC = BPC * T        # tokens per core = 1024
KC = HID // 128     # 16 contraction chunks

BF16 = mybir.dt.bfloat16
F32 = mybir.dt.float32

_PROGRAM = None


def _build_program():
    nc = bass.Bass("TRN2", target_bir_lowering=False, debug=False,
                   num_devices=N_CORES)

    hbT = nc.dram_tensor("hbT", [HID, TC], BF16, kind="ExternalInput")
    wqT = nc.dram_tensor("wqT", [HID, NH * D], BF16, kind="ExternalInput")
    wkT = nc.dram_tensor("wkT", [HID, NKV * D], BF16, kind="ExternalInput")
    wvT = nc.dram_tensor("wvT", [HID, NKV * D], BF16, kind="ExternalInput")
    woT = nc.dram_tensor("woT", [NH * D, HID], BF16, kind="ExternalInput")
    out = nc.dram_tensor("out", [HID, TC], F32, kind="ExternalOutput")

    QW = NH * D       # 2048
    KW = NKV * D      # 512
    VW = NKV * 2 * D  # 1024: per (b, sc) unit: 8 x [v_j (64) | ones (64)]

    with TileContext(nc) as tc:
        with tc.tile_pool(name="persist", bufs=1) as cpool:
            # kTd: kv head j duplicated on both partition halves:
            # kTd[p, 1024*j + 512*b + t], rows 0-63 and 64-127 both = kT_j
            kTd = cpool.tile([128, NKV * TC], BF16, tag="kTd")
            # v_sb[p, 4096*b + 1024*sc + 128*j + c]: c in 0..63 = v_j[s, c],
            # c in 64..127 = 1.0 (ones block -> PV matmul replicates the
            # softmax denominator across psum rows 64-127)
            v_sb = cpool.tile([128, BPC * 4 * VW], BF16, tag="v_sb")
            # dummy operand for PE warmup matmuls (HAM clock ramp); memset
            # first so the warmups are not stuck behind the 2MB v_sb memset
            warm_sb = cpool.tile([128, 512], BF16, tag="warm_sb")
            nc.gpsimd.memset(warm_sb[:, :], 0.0)
            nc.gpsimd.memset(v_sb[:, :], 1.0)

            # Per-k-chunk tiles, plain contiguous 2D DMAs: compute chases
            # the loads instead of waiting on one whole-tensor transfer.
            with tc.tile_pool(name="chunks", bufs=1) as ckpool:
                def load_hb_chunk(k):
                    t = ckpool.tile([128, TC], BF16, tag=f"hbk{k}",
                                    name=f"hbk{k}")
                    nc.sync.dma_start(out=t[:, :],
                                      in_=hbT[128 * k:128 * (k + 1), :])
                    return t

                def load_wv_chunk(wvpool, k):
                    t = wvpool.tile([128, KW], BF16, tag=f"wvk{k}",
                                    name=f"wvk{k}")
                    nc.sync.dma_start(out=t[:, :],
                                      in_=wvT[128 * k:128 * (k + 1), :])
                    return t

                def load_wk():
                    ts = []
                    for k in range(KC):
                        t = ckpool.tile([128, KW], BF16, tag=f"wkk{k}",
                                        name=f"wkk{k}")
                        nc.sync.dma_start(out=t[:, :],
                                          in_=wkT[128 * k:128 * (k + 1), :])
                        ts.append(t)
                    return ts

                def load_wq_quarter(q):
                    # alternating tags: quarter q's DMA waits only on
                    # quarter q-2's readers, so it prefetches one group
                    # ahead and overlaps the previous group's matmuls
                    ts = []
                    for k in range(KC):
                        t = ckpool.tile([128, QW // 4], BF16,
                                        tag=f"wq{'AB'[q % 2]}{k}",
                                        name=f"wq{q}_{k}")
                        nc.sync.dma_start(
                            out=t[:, :],
                            in_=wqT[128 * k:128 * (k + 1),
                                    (QW // 4) * q:(QW // 4) * (q + 1)])
                        ts.append(t)
                    return ts

                with tc.tile_pool(name="attn", bufs=1) as apool:
                    oTb = [apool.tile([128, KC * T], BF16, tag=f"oT{b}",
                                      name=f"oT{b}")
                           for b in range(BPC)]

                    # V projection in its own pools (released after use).
                    # DMA order interleaves wv with hb so V matmuls can chase
                    # the stream; k-outer over 8 live psum banks means the
                    # first matmul only needs chunk 0, not the whole tensor.
                    with tc.tile_pool(name="wvp", bufs=1) as wvpool:
                        hbk, wvk = [], []
                        for k in range(KC):
                            wvk.append(load_wv_chunk(wvpool, k))
                            hbk.append(load_hb_chunk(k))
                        wkk = load_wk()
                        wqk = load_wq_quarter(0)

                        # PE warmup: ~8 self-contained matmuls on zeros ramp
                        # the HAM clock gate to 2.4 GHz while the first DMA
                        # chunks are still in flight.
                        with tc.tile_pool(name="ps_warm", bufs=1,
                                          space="PSUM") as ps_warm:
                            wps = ps_warm.tile([128, 512], F32, tag="warm")
                            for _ in range(8):
                                nc.tensor.matmul(
                                    wps[:, :], lhsT=warm_sb[:, 0:128],
                                    rhs=warm_sb[:, :],
                                    start=True, stop=True,
                                )

                        with tc.tile_pool(name="ps_v", bufs=1,
                                          space="PSUM") as ps_v:
                            vps = [ps_v.tile([128, KW], F32, tag=f"psv{g}",
                                             name=f"psv{g}")
                                   for g in range(BPC * 4)]
                            for k in range(KC):
                                for g in range(BPC * 4):
                                    b, c = g // 4, g % 4
                                    nc.tensor.matmul(
                                        vps[g][:, :],
                                        lhsT=hbk[k][:, T * b + 128 * c:
                                                    T * b + 128 * c + 128],
                                        rhs=wvk[k][:, :],
                                        start=(k == 0), stop=(k == KC - 1),
                                    )
                            # drains split across DVE and ACT: serialized on
                            # one engine they are ~5.4us and the first k-proj
                            # psum write waits on the last drain (bank WAR)
                            for g in range(BPC * 4):
                                dst = v_sb[:, VW * g:VW * (g + 1)]
                                dst = dst.rearrange("p (j e) -> p j e",
                                                    e=2 * D)[:, :, 0:D]
                                src = vps[g][:, :].rearrange(
                                    "p (j d) -> p j d", d=D)
                                if g % 2 == 0:
                                    nc.vector.tensor_copy(dst, src)
                                else:
                                    nc.scalar.activation(
                                        dst, src,
                                        mybir.ActivationFunctionType.Copy)

                    with (
                        tc.tile_pool(name="ps_proj", bufs=2,
                                     space="PSUM") as ps_proj,
                        tc.tile_pool(name="wo", bufs=2) as wopool,
                    ):
                      def load_wo(mc):
                          wo = wopool.tile([128, KC * 128], BF16, tag="wo")
                          nc.sync.dma_start(
                              out=wo[:, :].rearrange("p (k m) -> p k m",
                                                     m=128),
                              in_=woT[:, 128 * mc:128 * (mc + 1)]
                              .rearrange("(k p) m -> p k m", p=128),
                          )
                          return wo

                      with (
                        tc.tile_pool(name="qTp", bufs=3) as qpool,
                        tc.tile_pool(name="pT", bufs=4) as ppool,
                        tc.tile_pool(name="lv", bufs=12) as lvpool,
                        tc.tile_pool(name="ps_st", bufs=1, space="PSUM") as ps_st,
                        tc.tile_pool(name="ps_pv", bufs=2, space="PSUM") as ps_pv,
                      ):
                        def k_proj(jc):
                            for b in range(BPC):
                                ps = ps_proj.tile([128, T], F32, tag="ps")
                                for k in range(KC):
                                    nc.tensor.matmul(
                                        ps[:, :],
                                        lhsT=wkk[k][:, 128 * jc:128 * jc + 128],
                                        rhs=hbk[k][:, T * b:T * (b + 1)],
                                        start=(k == 0), stop=(k == KC - 1),
                                    )
                                for j, lo in ((2 * jc, 0), (2 * jc + 1, 64)):
                                    src = ps[lo:lo + 64, :]
                                    nc.vector.tensor_copy(
                                        kTd[0:64,
                                            TC * j + T * b: TC * j + T * (b + 1)],
                                        src)
                                    nc.vector.tensor_copy(
                                        kTd[64:128,
                                            TC * j + T * b: TC * j + T * (b + 1)],
                                        src)

                        def attn_unit(pair, qTp, group_lvs):
                            j = pair // 2
                            for b in range(BPC):
                                pts = []
                                for scp in range(2):  # sc pairs
                                    st = ps_st.tile([128, 4 * T], F32, tag="st")
                                    for sci in range(2):
                                        sc = 2 * scp + sci
                                        for half in range(2):
                                            col = T * (2 * sci + half)
                                            nc.tensor.matmul(
                                                st[:, col:col + T],
                                                lhsT=kTd[64 * half:64 * half + 64,
                                                         TC * j + T * b + 128 * sc:
                                                         TC * j + T * b + 128 * sc + 128],
                                                rhs=qTp[64 * half:64 * half + 64,
                                                        T * b:T * (b + 1)],
                                                start=True, stop=True,
                                            )
                                    p_t = ppool.tile([128, 4 * T], BF16, tag="pT")
                                    ei = nc.scalar.activation(
                                        p_t[:, :], st[:, :],
                                        mybir.ActivationFunctionType.Exp,
                                    )
                                    attn_unit.last_exp = ei.ins
                                    pts.append(p_t)
                                pos = []
                                for half in range(2):
                                    po = ps_pv.tile([128, T], F32, tag="po")
                                    for sc in range(4):
                                        scp, sci = sc // 2, sc % 2
                                        col = T * (2 * sci + half)
                                        nc.tensor.matmul(
                                            po[:, :],
                                            lhsT=v_sb[:, VW * (4 * b + sc) + 128 * j:
                                                      VW * (4 * b + sc) + 128 * (j + 1)],
                                            rhs=pts[scp][:, col:col + T],
                                            start=(sc == 0), stop=(sc == 3),
                                        )
                                    pos.append(po)
                                # park denominators (both halves in one tile,
                                # rows matching oTb layout) and unnormalized
                                # o^T; psum frees immediately.
                                lv = lvpool.tile([128, T], F32, tag="lv")
                                nc.vector.tensor_copy(lv[0:64, :],
                                                      pos[0][64:128, :])
                                nc.vector.tensor_copy(lv[64:128, :],
                                                      pos[1][64:128, :])
                                nc.vector.tensor_copy(
                                    oTb[b][0:64, T * pair:T * (pair + 1)],
                                    pos[0][0:64, :])
                                nc.vector.tensor_copy(
                                    oTb[b][64:128, T * pair:T * (pair + 1)],
                                    pos[1][0:64, :])
                                group_lvs.append((pair, b, lv))

                        def finalize_group(group_lvs, last=False):
                            # batched reciprocals, in place. Each recip gets
                            # an explicit ordering dep on the group's LAST
                            # exp so the static schedule clusters them
                            # (2 ACT table swaps per group instead of 2 per
                            # head-pair -- the scheduler doesn't model table
                            # reload costs). Normalizes go on the idle GPSIMD
                            # engine: on DVE they head-of-line-block the qTp
                            # psum-drain casts the PE needs (observed 2.6us
                            # PE stalls at every group boundary). The last
                            # group's normalizes gate the O-projection, so
                            # they stay on the faster DVE (idle by then),
                            # ordered b0-first to unblock the first out rows.
                            last_exp = attn_unit.last_exp
                            ordered = sorted(group_lvs,
                                             key=lambda e: (e[1], e[0]))
                            for pair, b, lv in ordered:
                                ri = _act_reciprocal(nc, lv[:, :], lv[:, :])
                                _add_dep(ri.ins, last_exp,
                                         reason="cluster recips after exps")
                            eng = nc.vector if last else nc.gpsimd
                            for pair, b, lv in ordered:
                                eng.tensor_tensor(
                                    out=oTb[b][:, T * pair:T * (pair + 1)],
                                    in0=oTb[b][:, T * pair:T * (pair + 1)],
                                    in1=lv[:, :],
                                    op=mybir.AluOpType.mult,
                                )

                        # K-projection chunks feed attention just in time;
                        # ACT exp hides under projection matmuls.
                        prev_lvs = None
                        wo_pre = []
                        part_ps = []
                        for jc in range(4):
                            k_proj(jc)
                            if prev_lvs:
                                finalize_group(prev_lvs)
                            if jc < 3:
                                wqk_next = load_wq_quarter(jc + 1)
                            else:
                                # prefetch the first two Wo tiles so the
                                # output projection starts without a DMA
                                # bubble after the last attention unit
                                wo_pre = [load_wo(0), load_wo(1)]
                            group_lvs = []
                            for mq in range(4 * jc, 4 * jc + 4):
                                qTp = qpool.tile([128, TC], BF16, tag="qTp")
                                for b in range(BPC):
                                    ps = ps_proj.tile([128, T], F32, tag="ps")
                                    for k in range(KC):
                                        nc.tensor.matmul(
                                            ps[:, :],
                                            lhsT=wqk[k][:, 128 * (mq % 4):
                                                        128 * (mq % 4) + 128],
                                            rhs=hbk[k][:, T * b:T * (b + 1)],
                                            start=(k == 0), stop=(k == KC - 1),
                                        )
                                    nc.vector.tensor_copy(
                                        qTp[:, T * b:T * (b + 1)], ps[:, :])
                                if mq == 15:
                                    # partial O-projection chains (mc=0,
                                    # pairs 0..11, already normalized): PE
                                    # filler for the exp-latency stalls of
                                    # the last attention units, which have
                                    # no q-projection left to hide under.
                                    # ps_proj is free once mq15's q psums
                                    # drain; chains stay open and finish
                                    # after the last finalize.
                                    for b in range(BPC):
                                        ps = ps_proj.tile([128, T], F32,
                                                          tag="ps",
                                                          name=f"part{b}")
                                        for k in range(12):
                                            nc.tensor.matmul(
                                                ps[:, :],
                                                lhsT=wo_pre[0][:, 128 * k:
                                                               128 * k + 128],
                                                rhs=oTb[b][:, T * k:
                                                           T * (k + 1)],
                                                start=(k == 0), stop=False,
                                            )
                                        part_ps.append(ps)
                                attn_unit(mq, qTp, group_lvs)
                            prev_lvs = group_lvs
                            if jc < 3:
                                wqk = wqk_next
                        finalize_group(prev_lvs, last=True)

                      # ------------ output projection --------------------
                      with (
                        tc.tile_pool(name="outsb", bufs=2) as outpool,
                        tc.tile_pool(name="ps_wo", bufs=2, space="PSUM") as ps_wo,
                      ):
                        for mc in range(KC):
                            wo = wo_pre[mc] if mc < 2 else load_wo(mc)
                            for b in range(BPC):
                                if mc == 0:
                                    ps = part_ps[b]
                                    k0 = 12
                                else:
                                    ps = ps_wo.tile([128, T], F32, tag="psf")
                                    k0 = 0
                                for k in range(k0, KC):
                                    nc.tensor.matmul(
                                        ps[:, :],
                                        lhsT=wo[:, 128 * k:
                                                128 * k + 128],
                                        rhs=oTb[b][:, T * k:T * (k + 1)],
                                        start=(k == k0 and k0 == 0),
                                        stop=(k == KC - 1),
                                    )
                                osb = outpool.tile([128, T], F32, tag="osb")
                                nc.vector.tensor_copy(osb[:, :], ps[:, :])
                                nc.sync.dma_start(
                                    out=out[128 * mc:128 * (mc + 1),
                                            T * b:T * (b + 1)],
                                    in_=osb[:, :],
                                )

    _split_excess_waits(nc)
    return nc


def _get_program():
    global _PROGRAM
    if _PROGRAM is None:
        _PROGRAM = _build_program()
    return _PROGRAM


def _to_blocks_tokens(x):
    """[B, L, F] -> [NBLOCKS, T, F] with the reference's 3D block order."""
    Bn, L, F = x.shape
    n = GRID // BS
    x = x.reshape(Bn, n, BS, n, BS, n, BS, F)
    x = x.transpose(0, 1, 3, 5, 2, 4, 6, 7)
    return x.reshape(Bn * n * n * n, BS * BS * BS, F)


def _from_blocks_tokens(x):
    """[NBLOCKS, T, F] -> [B, L, F] inverse of _to_blocks_tokens."""
    NBf, Tf, F = x.shape
    n = GRID // BS
    x = x.reshape(B, n, n, n, BS, BS, BS, F)
    x = x.transpose(0, 1, 4, 2, 5, 3, 6, 7)
    return x.reshape(B, GRID * GRID * GRID, F)


def kernel(hidden_states, Wq, Wk, Wv, Wo, x_dim, y_dim, z_dim):
    hidden_states = np.asarray(hidden_states, dtype=np.float32)
    Wq = np.asarray(Wq, dtype=np.float32)
    Wk = np.asarray(Wk, dtype=np.float32)
    Wv = np.asarray(Wv, dtype=np.float32)
    Wo = np.asarray(Wo, dtype=np.float32)

    bf = ml_dtypes.bfloat16
    scale = 1.0 / np.sqrt(D)
    wqT = np.ascontiguousarray((Wq.T * scale).astype(bf))  # [HID, 2048]
    wkT = np.ascontiguousarray(Wk.T.astype(bf))            # [HID, 512]
    wvT = np.ascontiguousarray(Wv.T.astype(bf))            # [HID, 512]
    woT = np.ascontiguousarray(Wo.T.astype(bf))            # [2048, HID]

    blocks = _to_blocks_tokens(hidden_states)              # [16, 512, HID]

    in_maps = []
    for c in range(N_CORES):
        hb = blocks[BPC * c:BPC * (c + 1)]                 # [2, 512, HID]
        hbT = np.ascontiguousarray(
            hb.transpose(2, 0, 1).reshape(HID, TC).astype(bf)
        )
        in_maps.append({
            "hbT": hbT, "wqT": wqT, "wkT": wkT, "wvT": wvT, "woT": woT,
        })

    global _LAST_IN_MAPS
    _LAST_IN_MAPS = in_maps
    nc = _get_program()
    res = run_bass_kernel_spmd(nc, in_maps, list(range(N_CORES)))

    out_blocks = np.empty((NBLOCKS, T, HID), dtype=np.float32)
    for c in range(N_CORES):
        o = res.results[c]["out"]                          # [HID, 1024]
        for b in range(BPC):
            out_blocks[BPC * c + b] = o[:, T * b:T * (b + 1)].T
    return _from_blocks_tokens(out_blocks)



# revision 14
# speedup vs baseline: 1.0765x; 1.0765x over previous
"""Block-3D attention kernel for 8 Trainium2 NeuronCores.

Problem: B=2, 16x16x16 token grid, 8x8x8 blocks -> 16 independent blocks
of T=512 tokens. GQA attention (32 q heads, 8 kv heads, d=64) inside each
block, with QKV/O projections (hidden=2048).

Sharding: pure data-parallel over blocks - 2 blocks per core, full
weights replicated, no collectives. Each core runs an identical program
on its own slice.

Per-core dataflow (all matmuls bf16 with fp32 PSUM accumulation):
  hbT [2048,1024] (hidden, block-permuted, transposed, bf16)
  1. Q/K projections, weights stationary -> qT [2048,1024], kTdup
     (kv heads duplicated on both partition halves for 2-head row-tiled QK)
  2. V projection, activations stationary -> v [t, kv*64] (+ones cols)
  3. per (block, head-pair): st[s,t] = k q^T via two row-tiled matmuls;
     exp on ACT -> pT bf16
  4. PV: lhsT=pT chunks, rhs=[v|1] -> o[t, 65] psum; col 64 = sum(exp);
     vector reciprocal + per-partition tensor_scalar_mul -> o_all [t, hd]
  5. PE-transpose o_all -> oT [hd, t]
  6. Wo: lhsT=woT tiles, rhs=oT -> out^T [2048, 1024] f32
"""

import numpy as np
import ml_dtypes

import concourse.bass as bass
import concourse.mybir as mybir
from concourse.tile import TileContext
from concourse.masks import make_identity
from concourse.bass_utils import run_bass_kernel_spmd

# ---------------------------------------------------------------------------
# Workaround for this walrus build: at most 1 sync wait per Drain
# instruction, but TileContext's tail drain collects one wait per active
# proc. Split the waits across per-proc NOPs on the sync engine.
# ---------------------------------------------------------------------------
from concourse import tile as _tile
from concourse.vector_clock import ScopedClock as _ScopedClock
from concourse.vector_clock import VectorClock as _VectorClock
from concourse.tile_sem_assignment import N_PROCS as _N_PROCS


def _split_drain_and_barrier(self, tick_clock, wait_clock):
    gc = tick_clock.global_clock
    for p in range(_N_PROCS):
        if gc[p] == 0:
            continue
        c = _VectorClock([gc[q] if q == p else 0 for q in range(_N_PROCS)])
        nop = self.nc.sync.nop(nofuse=True)
        wait_clock.add_sem_waits(nop.ins, _ScopedClock({None: c}))
    # The NOPs above precede the drain in SP program order and carry all
    # required waits, so the drain itself needs none.
    self.nc.sync.drain()
    self.nc.all_engine_barrier()
    assert self.sems is not None
    popped = self.nc._tile_sem_poison_stack.pop()
    assert popped is self._sem_poison
    self.nc.clear_and_free_semaphores(list(self.sems.allocated().values()))
    self.nc.all_engine_barrier()


_tile.TileContext._drain_and_barrier = _split_drain_and_barrier

# This walrus also caps sync waits per regular instruction (observed: 3
# waits on a DVE TensorCopy rejected). Post-pass: move excess waits onto
# bass_nofuse NOPs inserted immediately before the instruction on the
# same engine.
_WAIT_CAP = 1

from concourse.tile_rust import add_dep_helper as _add_dep_helper


def _add_dep(from_inst, to_inst, reason=""):
    _add_dep_helper(from_inst, to_inst, sync=False, reason=reason)


def _act_reciprocal(nc, out, in_):
    """Reciprocal on the Scalar (ACT) engine. bass blocks
    ActivationFunctionType.Reciprocal for accuracy; measured on this HW the
    rel err is ~1.2e-5 for inputs in [300, 2500] (our softmax denominators),
    far below this kernel's bf16-dominated error floor, and it is ~5x
    cheaper than the exact DVE reciprocal at free size 512."""
    eng = nc.scalar
    return eng.add_instruction(
        mybir.InstActivation(
            name=nc.get_next_instruction_name(),
            func=mybir.ActivationFunctionType.Reciprocal,
            ins=[eng.lower_ap(in_),
                 mybir.ImmediateValue(dtype=mybir.dt.float32, value=0.0),
                 mybir.ImmediateValue(dtype=mybir.dt.float32, value=1.0),
                 mybir.ImmediateValue(dtype=mybir.dt.float32, value=0.0)],
            outs=[eng.lower_ap(out)],
        )
    )


def _split_excess_waits(nc, cap=_WAIT_CAP):
    count = 0
    for f in nc.m.functions:
        for bb in f.blocks:
            il = bb.instructions
            i = 0
            while i < len(il):
                inst = il[i]
                si = inst.sync_info
                c = 1 if isinstance(inst, mybir.InstDrain) else cap
                if si is not None and len(si.on_wait) > c:
                    waits = list(si.on_wait)
                    keep = waits[-c:] if c else []
                    excess = waits[:-c] if c else waits
                    pos = i
                    for g0 in range(0, len(excess), cap):
                        grp = excess[g0:g0 + cap]
                        count += 1
                        nop = mybir.InstNoOp(
                            name=f"waitsplit_{count}",
                            sync_info=mybir.SyncInfo(on_wait=grp, on_update=[]),
                            bass_nofuse=True,
                            engine=inst.engine,
                        )
                        il.insert(pos, nop)
                        pos += 1
                        i += 1
                    si.on_wait = keep
                i += 1
    return count

# ---------------------------------------------------------------------------
# Model constants (hardcoded per problem spec)
# ---------------------------------------------------------------------------
HID = 2048
NH = 32
NKV = 8
D = 64
B = 2
GRID = 16           # x_dim = y_dim = z_dim
BS = 8              # block size per axis
T = BS * BS * BS    # 512 tokens per block
NBLOCKS = 16        # total 3D blocks (B * 2*2*2)
N_CORES = 8
BPC = NBLOCKS // N_CORES  # blocks per core = 2
TC = BPC * T        # tokens per core = 1024
KC = HID // 128     # 16 contraction chunks

BF16 = mybir.dt.bfloat16
F32 = mybir.dt.float32

_PROGRAM = None


def _build_program():
    nc = bass.Bass("TRN2", target_bir_lowering=False, debug=False,
                   num_devices=N_CORES)

    hbT = nc.dram_tensor("hbT", [HID, TC], BF16, kind="ExternalInput")
    wqT = nc.dram_tensor("wqT", [HID, NH * D], BF16, kind="ExternalInput")
    wkT = nc.dram_tensor("wkT", [HID, NKV * D], BF16, kind="ExternalInput")
    wvT = nc.dram_tensor("wvT", [HID, NKV * D], BF16, kind="ExternalInput")
    woT = nc.dram_tensor("woT", [NH * D, HID], BF16, kind="ExternalInput")
    out = nc.dram_tensor("out", [HID, TC], F32, kind="ExternalOutput")

    QW = NH * D       # 2048
    KW = NKV * D      # 512
    VW = NKV * 2 * D  # 1024: per (b, sc) unit: 8 x [v_j (64) | ones (64)]

    with TileContext(nc) as tc:
        with tc.tile_pool(name="persist", bufs=1) as cpool:
            # kTd: kv head j duplicated on both partition halves:
            # kTd[p, 1024*j + 512*b + t], rows 0-63 and 64-127 both = kT_j
            kTd = cpool.tile([128, NKV * TC], BF16, tag="kTd")
            # v_sb[p, 4096*b + 1024*sc + 128*j + c]: c in 0..63 = v_j[s, c],
            # c in 64..127 = 1.0 (ones block -> PV matmul replicates the
            # softmax denominator across psum rows 64-127)
            v_sb = cpool.tile([128, BPC * 4 * VW], BF16, tag="v_sb")
            # dummy operand for PE warmup matmuls (HAM clock ramp); memset
            # first so the warmups are not stuck behind the 2MB v_sb memset
            warm_sb = cpool.tile([128, 512], BF16, tag="warm_sb")
            nc.gpsimd.memset(warm_sb[:, :], 0.0)
            nc.gpsimd.memset(v_sb[:, :], 1.0)

            # Per-k-chunk tiles, plain contiguous 2D DMAs: compute chases
            # the loads instead of waiting on one whole-tensor transfer.
            with tc.tile_pool(name="chunks", bufs=1) as ckpool:
                def load_hb_chunk(k):
                    t = ckpool.tile([128, TC], BF16, tag=f"hbk{k}",
                                    name=f"hbk{k}")
                    nc.sync.dma_start(out=t[:, :],
                                      in_=hbT[128 * k:128 * (k + 1), :])
                    return t

                def load_wv_chunk(wvpool, k):
                    t = wvpool.tile([128, KW], BF16, tag=f"wvk{k}",
                                    name=f"wvk{k}")
                    nc.sync.dma_start(out=t[:, :],
                                      in_=wvT[128 * k:128 * (k + 1), :])
                    return t

                def load_wk():
                    ts = []
                    for k in range(KC):
                        t = ckpool.tile([128, KW], BF16, tag=f"wkk{k}",
                                        name=f"wkk{k}")
                        nc.sync.dma_start(out=t[:, :],
                                          in_=wkT[128 * k:128 * (k + 1), :])
                        ts.append(t)
                    return ts

                def load_wq_quarter(q):
                    # alternating tags: quarter q's DMA waits only on
                    # quarter q-2's readers, so it prefetches one group
                    # ahead and overlaps the previous group's matmuls
                    ts = []
                    for k in range(KC):
                        t = ckpool.tile([128, QW // 4], BF16,
                                        tag=f"wq{'AB'[q % 2]}{k}",
                                        name=f"wq{q}_{k}")
                        nc.sync.dma_start(
                            out=t[:, :],
                            in_=wqT[128 * k:128 * (k + 1),
                                    (QW // 4) * q:(QW // 4) * (q + 1)])
                        ts.append(t)
                    return ts

                with tc.tile_pool(name="attn", bufs=1) as apool:
                    oTb = [apool.tile([128, KC * T], BF16, tag=f"oT{b}",
                                      name=f"oT{b}")
                           for b in range(BPC)]

                    # V projection in its own pools (released after use).
                    # DMA order interleaves wv with hb so V matmuls can chase
                    # the stream; k-outer over 8 live psum banks means the
                    # first matmul only needs chunk 0, not the whole tensor.
                    with tc.tile_pool(name="wvp", bufs=1) as wvpool:
                        hbk, wvk = [], []
                        for k in range(KC):
                            wvk.append(load_wv_chunk(wvpool, k))
                            hbk.append(load_hb_chunk(k))
                        wkk = load_wk()
                        wqk = load_wq_quarter(0)

                        # PE warmup: ~8 self-contained matmuls on zeros ramp
                        # the HAM clock gate to 2.4 GHz while the first DMA
                        # chunks are still in flight.
                        with tc.tile_pool(name="ps_warm", bufs=1,
                                          space="PSUM") as ps_warm:
                            wps = ps_warm.tile([128, 512], F32, tag="warm")
                            for _ in range(8):
                                nc.tensor.matmul(
                                    wps[:, :], lhsT=warm_sb[:, 0:128],
                                    rhs=warm_sb[:, :],
                                    start=True, stop=True,
                                )

                        with tc.tile_pool(name="ps_v", bufs=1,
                                          space="PSUM") as ps_v:
                            vps = [ps_v.tile([128, KW], F32, tag=f"psv{g}",
                                             name=f"psv{g}")
                                   for g in range(BPC * 4)]
                            for k in range(KC):
                                for g in range(BPC * 4):
                                    b, c = g // 4, g % 4
                                    nc.tensor.matmul(
                                        vps[g][:, :],
                                        lhsT=hbk[k][:, T * b + 128 * c:
                                                    T * b + 128 * c + 128],
                                        rhs=wvk[k][:, :],
                                        start=(k == 0), stop=(k == KC - 1),
                                    )
                            # drains split across DVE and ACT: serialized on
                            # one engine they are ~5.4us and the first k-proj
                            # psum write waits on the last drain (bank WAR)
                            for g in range(BPC * 4):
                                dst = v_sb[:, VW * g:VW * (g + 1)]
                                dst = dst.rearrange("p (j e) -> p j e",
                                                    e=2 * D)[:, :, 0:D]
                                src = vps[g][:, :].rearrange(
                                    "p (j d) -> p j d", d=D)
                                if g % 2 == 0:
                                    nc.vector.tensor_copy(dst, src)
                                else:
                                    nc.scalar.activation(
                                        dst, src,
                                        mybir.ActivationFunctionType.Copy)

                    with (
                        tc.tile_pool(name="ps_proj", bufs=2,
                                     space="PSUM") as ps_proj,
                        tc.tile_pool(name="wo", bufs=4) as wopool,
                    ):
                      def load_wo(mc):
                          wo = wopool.tile([128, KC * 128], BF16, tag="wo")
                          nc.sync.dma_start(
                              out=wo[:, :].rearrange("p (k m) -> p k m",
                                                     m=128),
                              in_=woT[:, 128 * mc:128 * (mc + 1)]
                              .rearrange("(k p) m -> p k m", p=128),
                          )
                          return wo

                      with (
                        tc.tile_pool(name="qTp", bufs=3) as qpool,
                        tc.tile_pool(name="pT", bufs=4) as ppool,
                        tc.tile_pool(name="lv", bufs=12) as lvpool,
                        tc.tile_pool(name="ps_st", bufs=1, space="PSUM") as ps_st,
                        tc.tile_pool(name="ps_pv", bufs=2, space="PSUM") as ps_pv,
                      ):
                        def k_proj(jc):
                            for b in range(BPC):
                                ps = ps_proj.tile([128, T], F32, tag="ps")
                                for k in range(KC):
                                    nc.tensor.matmul(
                                        ps[:, :],
                                        lhsT=wkk[k][:, 128 * jc:128 * jc + 128],
                                        rhs=hbk[k][:, T * b:T * (b + 1)],
                                        start=(k == 0), stop=(k == KC - 1),
                                    )
                                for j, lo in ((2 * jc, 0), (2 * jc + 1, 64)):
                                    src = ps[lo:lo + 64, :]
                                    nc.vector.tensor_copy(
                                        kTd[0:64,
                                            TC * j + T * b: TC * j + T * (b + 1)],
                                        src)
                                    nc.vector.tensor_copy(
                                        kTd[64:128,
                                            TC * j + T * b: TC * j + T * (b + 1)],
                                        src)

                        def attn_unit(pair, qTp, group_lvs):
                            j = pair // 2
                            for b in range(BPC):
                                pts = []
                                for scp in range(2):  # sc pairs
                                    st = ps_st.tile([128, 4 * T], F32, tag="st")
                                    for sci in range(2):
                                        sc = 2 * scp + sci
                                        for half in range(2):
                                            col = T * (2 * sci + half)
                                            nc.tensor.matmul(
                                                st[:, col:col + T],
                                                lhsT=kTd[64 * half:64 * half + 64,
                                                         TC * j + T * b + 128 * sc:
                                                         TC * j + T * b + 128 * sc + 128],
                                                rhs=qTp[64 * half:64 * half + 64,
                                                        T * b:T * (b + 1)],
                                                start=True, stop=True,
                                            )
                                    # exp split in two: halves the exp->PV
                                    # latency where no q-proj filler exists
                                    # (the tail units of the last group)
                                    p_t = ppool.tile([128, 4 * T], BF16, tag="pT")
                                    nc.scalar.activation(
                                        p_t[:, 0:2 * T], st[:, 0:2 * T],
                                        mybir.ActivationFunctionType.Exp,
                                    )
                                    ei = nc.scalar.activation(
                                        p_t[:, 2 * T:4 * T], st[:, 2 * T:4 * T],
                                        mybir.ActivationFunctionType.Exp,
                                    )
                                    attn_unit.last_exp = ei.ins
                                    pts.append(p_t)
                                pos = []
                                for half in range(2):
                                    po = ps_pv.tile([128, T], F32, tag="po")
                                    for sc in range(4):
                                        scp, sci = sc // 2, sc % 2
                                        col = T * (2 * sci + half)
                                        nc.tensor.matmul(
                                            po[:, :],
                                            lhsT=v_sb[:, VW * (4 * b + sc) + 128 * j:
                                                      VW * (4 * b + sc) + 128 * (j + 1)],
                                            rhs=pts[scp][:, col:col + T],
                                            start=(sc == 0), stop=(sc == 3),
                                        )
                                    pos.append(po)
                                # park denominators (both halves in one tile,
                                # rows matching oTb layout) and unnormalized
                                # o^T; psum frees immediately.
                                lv = lvpool.tile([128, T], F32, tag="lv")
                                nc.vector.tensor_copy(lv[0:64, :],
                                                      pos[0][64:128, :])
                                nc.vector.tensor_copy(lv[64:128, :],
                                                      pos[1][64:128, :])
                                nc.vector.tensor_copy(
                                    oTb[b][0:64, T * pair:T * (pair + 1)],
                                    pos[0][0:64, :])
                                nc.vector.tensor_copy(
                                    oTb[b][64:128, T * pair:T * (pair + 1)],
                                    pos[1][0:64, :])
                                group_lvs.append((pair, b, lv))

                        def finalize_group(group_lvs, last=False):
                            # batched reciprocals, in place. Each recip gets
                            # an explicit ordering dep on the group's LAST
                            # exp so the static schedule clusters them
                            # (2 ACT table swaps per group instead of 2 per
                            # head-pair -- the scheduler doesn't model table
                            # reload costs). Normalizes go on the idle GPSIMD
                            # engine: on DVE they head-of-line-block the qTp
                            # psum-drain casts the PE needs (observed 2.6us
                            # PE stalls at every group boundary). The last
                            # group's normalizes gate the O-projection, so
                            # they stay on the faster DVE (idle by then),
                            # ordered b0-first to unblock the first out rows.
                            last_exp = attn_unit.last_exp
                            ordered = sorted(group_lvs,
                                             key=lambda e: (e[1], e[0]))
                            for pair, b, lv in ordered:
                                ri = _act_reciprocal(nc, lv[:, :], lv[:, :])
                                _add_dep(ri.ins, last_exp,
                                         reason="cluster recips after exps")
                            eng = nc.vector if last else nc.gpsimd
                            for pair, b, lv in ordered:
                                eng.tensor_tensor(
                                    out=oTb[b][:, T * pair:T * (pair + 1)],
                                    in0=oTb[b][:, T * pair:T * (pair + 1)],
                                    in1=lv[:, :],
                                    op=mybir.AluOpType.mult,
                                )

                        # K-projection chunks feed attention just in time;
                        # ACT exp hides under projection matmuls.
                        prev_lvs = None
                        wo_pre = []
                        part_ps = []
                        for jc in range(4):
                            k_proj(jc)
                            if prev_lvs:
                                finalize_group(prev_lvs)
                            if jc < 3:
                                wqk_next = load_wq_quarter(jc + 1)
                            else:
                                # prefetch the first two Wo tiles so the
                                # output projection starts without a DMA
                                # bubble after the last attention unit
                                wo_pre = [load_wo(0), load_wo(1)]
                            group_lvs = []
                            for mq in range(4 * jc, 4 * jc + 4):
                                qTp = qpool.tile([128, TC], BF16, tag="qTp")
                                for b in range(BPC):
                                    ps = ps_proj.tile([128, T], F32, tag="ps")
                                    for k in range(KC):
                                        nc.tensor.matmul(
                                            ps[:, :],
                                            lhsT=wqk[k][:, 128 * (mq % 4):
                                                        128 * (mq % 4) + 128],
                                            rhs=hbk[k][:, T * b:T * (b + 1)],
                                            start=(k == 0), stop=(k == KC - 1),
                                        )
                                    nc.vector.tensor_copy(
                                        qTp[:, T * b:T * (b + 1)], ps[:, :])
                                if mq == 15:
                                    # partial O-projection chains (mc=0,
                                    # pairs 0..11, already normalized): PE
                                    # filler for the exp-latency stalls of
                                    # the last attention units, which have
                                    # no q-projection left to hide under.
                                    # ps_proj is free once mq15's q psums
                                    # drain; chains stay open and finish
                                    # after the last finalize. Prefetching
                                    # wo mc2/mc3 here also keeps the DMA
                                    # queue from going cold before the out
                                    # stores start.
                                    wo_pre += [load_wo(2), load_wo(3)]
                                    for b in range(BPC):
                                        ps = ps_proj.tile([128, T], F32,
                                                          tag="ps",
                                                          name=f"part{b}")
                                        for k in range(12):
                                            nc.tensor.matmul(
                                                ps[:, :],
                                                lhsT=wo_pre[0][:, 128 * k:
                                                               128 * k + 128],
                                                rhs=oTb[b][:, T * k:
                                                           T * (k + 1)],
                                                start=(k == 0), stop=False,
                                            )
                                        part_ps.append(ps)
                                attn_unit(mq, qTp, group_lvs)
                            prev_lvs = group_lvs
                            if jc < 3:
                                wqk = wqk_next
                        finalize_group(prev_lvs, last=True)

                      # ------------ output projection --------------------
                      # deep buffering (4 psums, 4 osb tiles, drains split
                      # DVE/ACT): the first stores hit a cold DMA queue with
                      # ~3us first-packet latency; with bufs=2 the osb-tile
                      # recycle fed that latency straight into the PE.
                      with (
                        tc.tile_pool(name="outsb", bufs=4) as outpool,
                        tc.tile_pool(name="ps_wo", bufs=4, space="PSUM") as ps_wo,
                      ):
                        for mc in range(KC):
                            wo = wo_pre[mc] if mc < 4 else load_wo(mc)
                            for b in range(BPC):
                                if mc == 0:
                                    ps = part_ps[b]
                                    k0 = 12
                                else:
                                    ps = ps_wo.tile([128, T], F32, tag="psf")
                                    k0 = 0
                                for k in range(k0, KC):
                                    nc.tensor.matmul(
                                        ps[:, :],
                                        lhsT=wo[:, 128 * k:
                                                128 * k + 128],
                                        rhs=oTb[b][:, T * k:T * (k + 1)],
                                        start=(k == k0 and k0 == 0),
                                        stop=(k == KC - 1),
                                    )
                                osb = outpool.tile([128, T], F32, tag="osb")
                                if (2 * mc + b) % 2 == 0:
                                    nc.vector.tensor_copy(osb[:, :], ps[:, :])
                                else:
                                    nc.scalar.activation(
                                        osb[:, :], ps[:, :],
                                        mybir.ActivationFunctionType.Copy)
                                nc.sync.dma_start(
                                    out=out[128 * mc:128 * (mc + 1),
                                            T * b:T * (b + 1)],
                                    in_=osb[:, :],
                                )

    _split_excess_waits(nc)
    return nc


def _get_program():
    global _PROGRAM
    if _PROGRAM is None:
        _PROGRAM = _build_program()
    return _PROGRAM


def _to_blocks_tokens(x):
    """[B, L, F] -> [NBLOCKS, T, F] with the reference's 3D block order."""
    Bn, L, F = x.shape
    n = GRID // BS
    x = x.reshape(Bn, n, BS, n, BS, n, BS, F)
    x = x.transpose(0, 1, 3, 5, 2, 4, 6, 7)
    return x.reshape(Bn * n * n * n, BS * BS * BS, F)


def _from_blocks_tokens(x):
    """[NBLOCKS, T, F] -> [B, L, F] inverse of _to_blocks_tokens."""
    NBf, Tf, F = x.shape
    n = GRID // BS
    x = x.reshape(B, n, n, n, BS, BS, BS, F)
    x = x.transpose(0, 1, 4, 2, 5, 3, 6, 7)
    return x.reshape(B, GRID * GRID * GRID, F)


def kernel(hidden_states, Wq, Wk, Wv, Wo, x_dim, y_dim, z_dim):
    hidden_states = np.asarray(hidden_states, dtype=np.float32)
    Wq = np.asarray(Wq, dtype=np.float32)
    Wk = np.asarray(Wk, dtype=np.float32)
    Wv = np.asarray(Wv, dtype=np.float32)
    Wo = np.asarray(Wo, dtype=np.float32)

    bf = ml_dtypes.bfloat16
    scale = 1.0 / np.sqrt(D)
    wqT = np.ascontiguousarray((Wq.T * scale).astype(bf))  # [HID, 2048]
    wkT = np.ascontiguousarray(Wk.T.astype(bf))            # [HID, 512]
    wvT = np.ascontiguousarray(Wv.T.astype(bf))            # [HID, 512]
    woT = np.ascontiguousarray(Wo.T.astype(bf))            # [2048, HID]

    blocks = _to_blocks_tokens(hidden_states)              # [16, 512, HID]

    in_maps = []
    for c in range(N_CORES):
        hb = blocks[BPC * c:BPC * (c + 1)]                 # [2, 512, HID]
        hbT = np.ascontiguousarray(
            hb.transpose(2, 0, 1).reshape(HID, TC).astype(bf)
        )
        in_maps.append({
            "hbT": hbT, "wqT": wqT, "wkT": wkT, "wvT": wvT, "woT": woT,
        })

    global _LAST_IN_MAPS
    _LAST_IN_MAPS = in_maps
    nc = _get_program()
    res = run_bass_kernel_spmd(nc, in_maps, list(range(N_CORES)))

    out_blocks = np.empty((NBLOCKS, T, HID), dtype=np.float32)
    for c in range(N_CORES):
        o = res.results[c]["out"]                          # [HID, 1024]
        for b in range(BPC):
            out_blocks[BPC * c + b] = o[:, T * b:T * (b + 1)].T
    return _from_blocks_tokens(out_blocks)



# revision 18
# speedup vs baseline: 1.1197x; 1.0401x over previous
"""Block-3D attention kernel for 8 Trainium2 NeuronCores.

Problem: B=2, 16x16x16 token grid, 8x8x8 blocks -> 16 independent blocks
of T=512 tokens. GQA attention (32 q heads, 8 kv heads, d=64) inside each
block, with QKV/O projections (hidden=2048).

Sharding: pure data-parallel over blocks - 2 blocks per core, full
weights replicated, no collectives. Each core runs an identical program
on its own slice.

Per-core dataflow (all matmuls bf16 with fp32 PSUM accumulation):
  hbT [2048,1024] (hidden, block-permuted, transposed, bf16)
  1. Q/K projections, weights stationary -> qT [2048,1024], kTdup
     (kv heads duplicated on both partition halves for 2-head row-tiled QK)
  2. V projection, activations stationary -> v [t, kv*64] (+ones cols)
  3. per (block, head-pair): st[s,t] = k q^T via two row-tiled matmuls;
     exp on ACT -> pT bf16
  4. PV: lhsT=pT chunks, rhs=[v|1] -> o[t, 65] psum; col 64 = sum(exp);
     vector reciprocal + per-partition tensor_scalar_mul -> o_all [t, hd]
  5. PE-transpose o_all -> oT [hd, t]
  6. Wo: lhsT=woT tiles, rhs=oT -> out^T [2048, 1024] f32
"""

import numpy as np
import ml_dtypes

import concourse.bass as bass
import concourse.mybir as mybir
from concourse.tile import TileContext
from concourse.masks import make_identity
from concourse.bass_utils import run_bass_kernel_spmd

# ---------------------------------------------------------------------------
# Workaround for this walrus build: at most 1 sync wait per Drain
# instruction, but TileContext's tail drain collects one wait per active
# proc. Split the waits across per-proc NOPs on the sync engine.
# ---------------------------------------------------------------------------
from concourse import tile as _tile
from concourse.vector_clock import ScopedClock as _ScopedClock
from concourse.vector_clock import VectorClock as _VectorClock
from concourse.tile_sem_assignment import N_PROCS as _N_PROCS


def _split_drain_and_barrier(self, tick_clock, wait_clock):
    gc = tick_clock.global_clock
    for p in range(_N_PROCS):
        if gc[p] == 0:
            continue
        c = _VectorClock([gc[q] if q == p else 0 for q in range(_N_PROCS)])
        nop = self.nc.sync.nop(nofuse=True)
        wait_clock.add_sem_waits(nop.ins, _ScopedClock({None: c}))
    # The NOPs above precede the drain in SP program order and carry all
    # required waits, so the drain itself needs none.
    self.nc.sync.drain()
    self.nc.all_engine_barrier()
    assert self.sems is not None
    popped = self.nc._tile_sem_poison_stack.pop()
    assert popped is self._sem_poison
    self.nc.clear_and_free_semaphores(list(self.sems.allocated().values()))
    self.nc.all_engine_barrier()


_tile.TileContext._drain_and_barrier = _split_drain_and_barrier

# This walrus also caps sync waits per regular instruction (observed: 3
# waits on a DVE TensorCopy rejected). Post-pass: move excess waits onto
# bass_nofuse NOPs inserted immediately before the instruction on the
# same engine.
_WAIT_CAP = 1

from concourse.tile_rust import add_dep_helper as _add_dep_helper


def _add_dep(from_inst, to_inst, reason=""):
    _add_dep_helper(from_inst, to_inst, sync=False, reason=reason)


def _act_reciprocal(nc, out, in_):
    """Reciprocal on the Scalar (ACT) engine. bass blocks
    ActivationFunctionType.Reciprocal for accuracy; measured on this HW the
    rel err is ~1.2e-5 for inputs in [300, 2500] (our softmax denominators),
    far below this kernel's bf16-dominated error floor, and it is ~5x
    cheaper than the exact DVE reciprocal at free size 512."""
    eng = nc.scalar
    return eng.add_instruction(
        mybir.InstActivation(
            name=nc.get_next_instruction_name(),
            func=mybir.ActivationFunctionType.Reciprocal,
            ins=[eng.lower_ap(in_),
                 mybir.ImmediateValue(dtype=mybir.dt.float32, value=0.0),
                 mybir.ImmediateValue(dtype=mybir.dt.float32, value=1.0),
                 mybir.ImmediateValue(dtype=mybir.dt.float32, value=0.0)],
            outs=[eng.lower_ap(out)],
        )
    )


def _split_excess_waits(nc, cap=_WAIT_CAP):
    count = 0
    for f in nc.m.functions:
        for bb in f.blocks:
            il = bb.instructions
            i = 0
            while i < len(il):
                inst = il[i]
                si = inst.sync_info
                c = 1 if isinstance(inst, mybir.InstDrain) else cap
                if si is not None and len(si.on_wait) > c:
                    waits = list(si.on_wait)
                    keep = waits[-c:] if c else []
                    excess = waits[:-c] if c else waits
                    pos = i
                    for g0 in range(0, len(excess), cap):
                        grp = excess[g0:g0 + cap]
                        count += 1
                        nop = mybir.InstNoOp(
                            name=f"waitsplit_{count}",
                            sync_info=mybir.SyncInfo(on_wait=grp, on_update=[]),
                            bass_nofuse=True,
                            engine=inst.engine,
                        )
                        il.insert(pos, nop)
                        pos += 1
                        i += 1
                    si.on_wait = keep
                i += 1
    return count

# ---------------------------------------------------------------------------
# Model constants (hardcoded per problem spec)
# ---------------------------------------------------------------------------
HID = 2048
NH = 32
NKV = 8
D = 64
B = 2
GRID = 16           # x_dim = y_dim = z_dim
BS = 8              # block size per axis
T = BS * BS * BS    # 512 tokens per block
NBLOCKS = 16        # total 3D blocks (B * 2*2*2)
N_CORES = 8
BPC = NBLOCKS // N_CORES  # blocks per core = 2
TC = BPC * T        # tokens per core = 1024
KC = HID // 128     # 16 contraction chunks

BF16 = mybir.dt.bfloat16
F32 = mybir.dt.float32

_PROGRAM = None


def _build_program():
    nc = bass.Bass("TRN2", target_bir_lowering=False, debug=False,
                   num_devices=N_CORES)

    hbT = nc.dram_tensor("hbT", [HID, TC], BF16, kind="ExternalInput")
    wqT = nc.dram_tensor("wqT", [HID, NH * D], BF16, kind="ExternalInput")
    wkT = nc.dram_tensor("wkT", [HID, NKV * D], BF16, kind="ExternalInput")
    wvT = nc.dram_tensor("wvT", [HID, NKV * D], BF16, kind="ExternalInput")
    woT = nc.dram_tensor("woT", [NH * D, HID], BF16, kind="ExternalInput")
    out = nc.dram_tensor("out", [HID, TC], F32, kind="ExternalOutput")

    QW = NH * D       # 2048
    KW = NKV * D      # 512
    VW = NKV * 2 * D  # 1024: per (b, sc) unit: 8 x [v_j (64) | ones (64)]

    with TileContext(nc) as tc:
        with tc.tile_pool(name="persist", bufs=1) as cpool:
            # kTd: kv head j duplicated on both partition halves:
            # kTd[p, 1024*j + 512*b + t], rows 0-63 and 64-127 both = kT_j
            kTd = cpool.tile([128, NKV * TC], BF16, tag="kTd")
            # v_sb[p, 4096*b + 1024*sc + 128*j + c]: c in 0..63 = v_j[s, c],
            # c in 64..127 = 1.0 (ones block -> PV matmul replicates the
            # softmax denominator across psum rows 64-127)
            v_sb = cpool.tile([128, BPC * 4 * VW], BF16, tag="v_sb")
            # dummy operand for PE warmup matmuls (HAM clock ramp); memset
            # first so the warmups are not stuck behind the 2MB v_sb memset
            warm_sb = cpool.tile([128, 512], BF16, tag="warm_sb")
            nc.gpsimd.memset(warm_sb[:, :], 0.0)
            nc.gpsimd.memset(v_sb[:, :], 1.0)

            # Per-k-chunk tiles, plain contiguous 2D DMAs: compute chases
            # the loads instead of waiting on one whole-tensor transfer.
            with tc.tile_pool(name="chunks", bufs=1) as ckpool:
                def load_hb_chunk(k):
                    t = ckpool.tile([128, TC], BF16, tag=f"hbk{k}",
                                    name=f"hbk{k}")
                    nc.sync.dma_start(out=t[:, :],
                                      in_=hbT[128 * k:128 * (k + 1), :])
                    return t

                def load_wv_chunk(wvpool, k):
                    t = wvpool.tile([128, KW], BF16, tag=f"wvk{k}",
                                    name=f"wvk{k}")
                    nc.sync.dma_start(out=t[:, :],
                                      in_=wvT[128 * k:128 * (k + 1), :])
                    return t

                def load_wk():
                    ts = []
                    for k in range(KC):
                        t = ckpool.tile([128, KW], BF16, tag=f"wkk{k}",
                                        name=f"wkk{k}")
                        nc.sync.dma_start(out=t[:, :],
                                          in_=wkT[128 * k:128 * (k + 1), :])
                        ts.append(t)
                    return ts

                def load_wq_quarter(q):
                    # alternating tags: quarter q's DMA waits only on
                    # quarter q-2's readers, so it prefetches one group
                    # ahead and overlaps the previous group's matmuls
                    ts = []
                    for k in range(KC):
                        t = ckpool.tile([128, QW // 4], BF16,
                                        tag=f"wq{'AB'[q % 2]}{k}",
                                        name=f"wq{q}_{k}")
                        nc.sync.dma_start(
                            out=t[:, :],
                            in_=wqT[128 * k:128 * (k + 1),
                                    (QW // 4) * q:(QW // 4) * (q + 1)])
                        ts.append(t)
                    return ts

                with tc.tile_pool(name="attn", bufs=1) as apool:
                    oTb = [apool.tile([128, KC * T], BF16, tag=f"oT{b}",
                                      name=f"oT{b}")
                           for b in range(BPC)]

                    # V projection in its own pools (released after use).
                    # DMA order interleaves wv with hb so V matmuls can chase
                    # the stream; k-outer over 8 live psum banks means the
                    # first matmul only needs chunk 0, not the whole tensor.
                    with tc.tile_pool(name="wvp", bufs=1) as wvpool:
                        hbk, wvk = [], []
                        for k in range(KC):
                            wvk.append(load_wv_chunk(wvpool, k))
                            hbk.append(load_hb_chunk(k))
                        wkk = load_wk()
                        wqk = load_wq_quarter(0)

                        # PE warmup: ~8 self-contained matmuls on zeros ramp
                        # the HAM clock gate to 2.4 GHz while the first DMA
                        # chunks are still in flight.
                        with tc.tile_pool(name="ps_warm", bufs=1,
                                          space="PSUM") as ps_warm:
                            wps = ps_warm.tile([128, 512], F32, tag="warm")
                            for _ in range(8):
                                nc.tensor.matmul(
                                    wps[:, :], lhsT=warm_sb[:, 0:128],
                                    rhs=warm_sb[:, :],
                                    start=True, stop=True,
                                )

                        with tc.tile_pool(name="ps_v", bufs=1,
                                          space="PSUM") as ps_v:
                            vps = [ps_v.tile([128, KW], F32, tag=f"psv{g}",
                                             name=f"psv{g}")
                                   for g in range(BPC * 4)]
                            for k in range(KC):
                                for g in range(BPC * 4):
                                    b, c = g // 4, g % 4
                                    nc.tensor.matmul(
                                        vps[g][:, :],
                                        lhsT=hbk[k][:, T * b + 128 * c:
                                                    T * b + 128 * c + 128],
                                        rhs=wvk[k][:, :],
                                        start=(k == 0), stop=(k == KC - 1),
                                    )
                            # drains split across DVE and ACT: serialized on
                            # one engine they are ~5.4us and the first k-proj
                            # psum write waits on the last drain (bank WAR)
                            for g in range(BPC * 4):
                                dst = v_sb[:, VW * g:VW * (g + 1)]
                                dst = dst.rearrange("p (j e) -> p j e",
                                                    e=2 * D)[:, :, 0:D]
                                src = vps[g][:, :].rearrange(
                                    "p (j d) -> p j d", d=D)
                                if g % 2 == 0:
                                    nc.vector.tensor_copy(dst, src)
                                else:
                                    nc.scalar.activation(
                                        dst, src,
                                        mybir.ActivationFunctionType.Copy)

                    with (
                        tc.tile_pool(name="ps_proj", bufs=2,
                                     space="PSUM") as ps_proj,
                        tc.tile_pool(name="wo", bufs=4) as wopool,
                    ):
                      def load_wo(mc):
                          wo = wopool.tile([128, KC * 128], BF16, tag="wo")
                          nc.sync.dma_start(
                              out=wo[:, :].rearrange("p (k m) -> p k m",
                                                     m=128),
                              in_=woT[:, 128 * mc:128 * (mc + 1)]
                              .rearrange("(k p) m -> p k m", p=128),
                          )
                          return wo

                      with (
                        tc.tile_pool(name="qTp", bufs=3) as qpool,
                        tc.tile_pool(name="pT", bufs=5) as ppool,
                        tc.tile_pool(name="lv", bufs=10) as lvpool,
                        tc.tile_pool(name="ps_st", bufs=1, space="PSUM") as ps_st,
                        tc.tile_pool(name="ps_pv", bufs=2, space="PSUM") as ps_pv,
                      ):
                        def k_proj(jc):
                            for b in range(BPC):
                                ps = ps_proj.tile([128, T], F32, tag="ps")
                                for k in range(KC):
                                    nc.tensor.matmul(
                                        ps[:, :],
                                        lhsT=wkk[k][:, 128 * jc:128 * jc + 128],
                                        rhs=hbk[k][:, T * b:T * (b + 1)],
                                        start=(k == 0), stop=(k == KC - 1),
                                    )
                                for j, lo in ((2 * jc, 0), (2 * jc + 1, 64)):
                                    src = ps[lo:lo + 64, :]
                                    nc.vector.tensor_copy(
                                        kTd[0:64,
                                            TC * j + T * b: TC * j + T * (b + 1)],
                                        src)
                                    nc.vector.tensor_copy(
                                        kTd[64:128,
                                            TC * j + T * b: TC * j + T * (b + 1)],
                                        src)

                        def attn_unit(pair, qTp, group_lvs):
                            j = pair // 2
                            for b in range(BPC):
                                pts = []
                                for scp in range(2):  # sc pairs
                                    st = ps_st.tile([128, 4 * T], F32, tag="st")
                                    for sci in range(2):
                                        sc = 2 * scp + sci
                                        for half in range(2):
                                            col = T * (2 * sci + half)
                                            nc.tensor.matmul(
                                                st[:, col:col + T],
                                                lhsT=kTd[64 * half:64 * half + 64,
                                                         TC * j + T * b + 128 * sc:
                                                         TC * j + T * b + 128 * sc + 128],
                                                rhs=qTp[64 * half:64 * half + 64,
                                                        T * b:T * (b + 1)],
                                                start=True, stop=True,
                                            )
                                    # exp split in two: halves the exp->PV
                                    # latency where no q-proj filler exists
                                    # (the tail units of the last group)
                                    p_t = ppool.tile([128, 4 * T], BF16, tag="pT")
                                    nc.scalar.activation(
                                        p_t[:, 0:2 * T], st[:, 0:2 * T],
                                        mybir.ActivationFunctionType.Exp,
                                    )
                                    ei = nc.scalar.activation(
                                        p_t[:, 2 * T:4 * T], st[:, 2 * T:4 * T],
                                        mybir.ActivationFunctionType.Exp,
                                    )
                                    attn_unit.last_exp = ei.ins
                                    pts.append(p_t)
                                # PV matmuls demoted below the NEXT unit's
                                # st matmuls: a PV op whose exp retires mid
                                # st-pair would otherwise preempt the pair
                                # and break its row-tiled 2x overlap (64 of
                                # 256 st MMs lost pairing, ~12us).
                                pos = []
                                with tc.high_priority(offset=-60):
                                    for half in range(2):
                                        po = ps_pv.tile([128, T], F32, tag="po")
                                        for sc in range(4):
                                            scp, sci = sc // 2, sc % 2
                                            col = T * (2 * sci + half)
                                            nc.tensor.matmul(
                                                po[:, :],
                                                lhsT=v_sb[:, VW * (4 * b + sc) + 128 * j:
                                                          VW * (4 * b + sc) + 128 * (j + 1)],
                                                rhs=pts[scp][:, col:col + T],
                                                start=(sc == 0), stop=(sc == 3),
                                            )
                                        pos.append(po)
                                # park denominators (both halves in one tile,
                                # rows matching oTb layout) and unnormalized
                                # o^T; psum frees immediately.
                                lv = lvpool.tile([128, T], F32, tag="lv")
                                nc.vector.tensor_copy(lv[0:64, :],
                                                      pos[0][64:128, :])
                                nc.vector.tensor_copy(lv[64:128, :],
                                                      pos[1][64:128, :])
                                nc.vector.tensor_copy(
                                    oTb[b][0:64, T * pair:T * (pair + 1)],
                                    pos[0][0:64, :])
                                nc.vector.tensor_copy(
                                    oTb[b][64:128, T * pair:T * (pair + 1)],
                                    pos[1][0:64, :])
                                group_lvs.append((pair, b, lv))

                        def finalize_group(group_lvs, last=False):
                            # batched reciprocals, in place. Each recip gets
                            # an explicit ordering dep on the group's LAST
                            # exp so the static schedule clusters them
                            # (2 ACT table swaps per group instead of 2 per
                            # head-pair -- the scheduler doesn't model table
                            # reload costs). Normalizes go on the idle GPSIMD
                            # engine: on DVE they head-of-line-block the qTp
                            # psum-drain casts the PE needs (observed 2.6us
                            # PE stalls at every group boundary). The last
                            # group's normalizes gate the O-projection, so
                            # they stay on the faster DVE (idle by then),
                            # ordered b0-first to unblock the first out rows.
                            last_exp = attn_unit.last_exp
                            ordered = sorted(group_lvs,
                                             key=lambda e: (e[1], e[0]))
                            for pair, b, lv in ordered:
                                ri = _act_reciprocal(nc, lv[:, :], lv[:, :])
                                _add_dep(ri.ins, last_exp,
                                         reason="cluster recips after exps")
                            eng = nc.vector if last else nc.gpsimd
                            for pair, b, lv in ordered:
                                eng.tensor_tensor(
                                    out=oTb[b][:, T * pair:T * (pair + 1)],
                                    in0=oTb[b][:, T * pair:T * (pair + 1)],
                                    in1=lv[:, :],
                                    op=mybir.AluOpType.mult,
                                )

                        # K-projection chunks feed attention just in time;
                        # ACT exp hides under projection matmuls.
                        prev_lvs = None
                        wo_pre = []
                        part_ps = []
                        for jc in range(4):
                            k_proj(jc)
                            if prev_lvs:
                                finalize_group(prev_lvs)
                            if jc < 3:
                                wqk_next = load_wq_quarter(jc + 1)
                            else:
                                # prefetch the first two Wo tiles so the
                                # output projection starts without a DMA
                                # bubble after the last attention unit
                                wo_pre = [load_wo(0), load_wo(1)]
                            group_lvs = []
                            for mq in range(4 * jc, 4 * jc + 4):
                                qTp = qpool.tile([128, TC], BF16, tag="qTp")
                                for b in range(BPC):
                                    ps = ps_proj.tile([128, T], F32, tag="ps")
                                    for k in range(KC):
                                        nc.tensor.matmul(
                                            ps[:, :],
                                            lhsT=wqk[k][:, 128 * (mq % 4):
                                                        128 * (mq % 4) + 128],
                                            rhs=hbk[k][:, T * b:T * (b + 1)],
                                            start=(k == 0), stop=(k == KC - 1),
                                        )
                                    nc.vector.tensor_copy(
                                        qTp[:, T * b:T * (b + 1)], ps[:, :])
                                if mq == 15:
                                    # partial O-projection chains (mc=0,
                                    # pairs 0..11, already normalized): PE
                                    # filler for the exp-latency stalls of
                                    # the last attention units, which have
                                    # no q-projection left to hide under.
                                    # ps_proj is free once mq15's q psums
                                    # drain; chains stay open and finish
                                    # after the last finalize. Prefetching
                                    # wo mc2/mc3 here also keeps the DMA
                                    # queue from going cold before the out
                                    # stores start.
                                    wo_pre += [load_wo(2), load_wo(3)]
                                    with tc.high_priority(offset=-600):
                                        for b in range(BPC):
                                            ps = ps_proj.tile([128, T], F32,
                                                              tag="ps",
                                                              name=f"part{b}")
                                            for k in range(12):
                                                nc.tensor.matmul(
                                                    ps[:, :],
                                                    lhsT=wo_pre[0][:, 128 * k:
                                                                   128 * k + 128],
                                                    rhs=oTb[b][:, T * k:
                                                               T * (k + 1)],
                                                    start=(k == 0), stop=False,
                                                )
                                            part_ps.append(ps)
                                attn_unit(mq, qTp, group_lvs)
                            prev_lvs = group_lvs
                            if jc < 3:
                                wqk = wqk_next
                        finalize_group(prev_lvs, last=True)

                      # ------------ output projection --------------------
                      # deep buffering (4 psums, 4 osb tiles, drains split
                      # DVE/ACT): the first stores hit a cold DMA queue with
                      # ~3us first-packet latency; with bufs=2 the osb-tile
                      # recycle fed that latency straight into the PE.
                      with (
                        tc.tile_pool(name="outsb", bufs=4) as outpool,
                        tc.tile_pool(name="ps_wo", bufs=4, space="PSUM") as ps_wo,
                      ):
                        for mc in range(KC):
                            wo = wo_pre[mc] if mc < 4 else load_wo(mc)
                            for b in range(BPC):
                                if mc == 0:
                                    ps = part_ps[b]
                                    k0 = 12
                                else:
                                    ps = ps_wo.tile([128, T], F32, tag="psf")
                                    k0 = 0
                                for k in range(k0, KC):
                                    nc.tensor.matmul(
                                        ps[:, :],
                                        lhsT=wo[:, 128 * k:
                                                128 * k + 128],
                                        rhs=oTb[b][:, T * k:T * (k + 1)],
                                        start=(k == k0 and k0 == 0),
                                        stop=(k == KC - 1),
                                    )
                                osb = outpool.tile([128, T], F32, tag="osb")
                                if (2 * mc + b) % 2 == 0:
                                    nc.vector.tensor_copy(osb[:, :], ps[:, :])
                                else:
                                    nc.scalar.activation(
                                        osb[:, :], ps[:, :],
                                        mybir.ActivationFunctionType.Copy)
                                nc.sync.dma_start(
                                    out=out[128 * mc:128 * (mc + 1),
                                            T * b:T * (b + 1)],
                                    in_=osb[:, :],
                                )

    _split_excess_waits(nc)
    return nc


def _get_program():
    global _PROGRAM
    if _PROGRAM is None:
        _PROGRAM = _build_program()
    return _PROGRAM


def _to_blocks_tokens(x):
    """[B, L, F] -> [NBLOCKS, T, F] with the reference's 3D block order."""
    Bn, L, F = x.shape
    n = GRID // BS
    x = x.reshape(Bn, n, BS, n, BS, n, BS, F)
    x = x.transpose(0, 1, 3, 5, 2, 4, 6, 7)
    return x.reshape(Bn * n * n * n, BS * BS * BS, F)


def _from_blocks_tokens(x):
    """[NBLOCKS, T, F] -> [B, L, F] inverse of _to_blocks_tokens."""
    NBf, Tf, F = x.shape
    n = GRID // BS
    x = x.reshape(B, n, n, n, BS, BS, BS, F)
    x = x.transpose(0, 1, 4, 2, 5, 3, 6, 7)
    return x.reshape(B, GRID * GRID * GRID, F)


def kernel(hidden_states, Wq, Wk, Wv, Wo, x_dim, y_dim, z_dim):
    hidden_states = np.asarray(hidden_states, dtype=np.float32)
    Wq = np.asarray(Wq, dtype=np.float32)
    Wk = np.asarray(Wk, dtype=np.float32)
    Wv = np.asarray(Wv, dtype=np.float32)
    Wo = np.asarray(Wo, dtype=np.float32)

    bf = ml_dtypes.bfloat16
    scale = 1.0 / np.sqrt(D)
    wqT = np.ascontiguousarray((Wq.T * scale).astype(bf))  # [HID, 2048]
    wkT = np.ascontiguousarray(Wk.T.astype(bf))            # [HID, 512]
    wvT = np.ascontiguousarray(Wv.T.astype(bf))            # [HID, 512]
    woT = np.ascontiguousarray(Wo.T.astype(bf))            # [2048, HID]

    blocks = _to_blocks_tokens(hidden_states)              # [16, 512, HID]

    in_maps = []
    for c in range(N_CORES):
        hb = blocks[BPC * c:BPC * (c + 1)]                 # [2, 512, HID]
        hbT = np.ascontiguousarray(
            hb.transpose(2, 0, 1).reshape(HID, TC).astype(bf)
        )
        in_maps.append({
            "hbT": hbT, "wqT": wqT, "wkT": wkT, "wvT": wvT, "woT": woT,
        })

    global _LAST_IN_MAPS
    _LAST_IN_MAPS = in_maps
    nc = _get_program()
    res = run_bass_kernel_spmd(nc, in_maps, list(range(N_CORES)))

    out_blocks = np.empty((NBLOCKS, T, HID), dtype=np.float32)
    for c in range(N_CORES):
        o = res.results[c]["out"]                          # [HID, 1024]
        for b in range(BPC):
            out_blocks[BPC * c + b] = o[:, T * b:T * (b + 1)].T
    return _from_blocks_tokens(out_blocks)

